# revision 1
# baseline (speedup 1.0000x reference)
"""BiCutLoss Trainium2 kernel (nn_BiCutLoss_52312701665760).

Reference computation (per batch row i of output[B, L, 2], labels[B, L]):
  temp = argmax(output, -1)            # 1 iff out1 > out0
  cut  = L if all(temp == 1) else (index of last 0 in temp)
  mask = arange(L) < cut
  r1   = where(labels == 1, -3.6/log2(j+2), 0.065)
  loss = sum(out1 * mask * r1) / B

Kernel formulation (exactly equivalent):
  d[j] = out0[j] - out1[j]                       # temp[j]==0  <=>  d[j] >= 0
  M[j] = max(d[j:], -1)  (reverse cummax; M[L] = -1 pad)
  thr  = 0 if M[0] >= 0 else -BIG                # all-ones row => mask all 1
  mask[j] = (M[j+1] >= thr)
  A_i = sum_j out1*mask          B_i = sum_j out1*mask*labels*(r1pos - 0.065)
  loss_i = 0.065*A_i + B_i

Sharding: pure data parallel — B=4096 rows split as 512 rows x 8 cores; each
core computes per-row partials [128,1] (4 row-tiles of 128 partitions), host
sums and divides by B.

Engines per [128, 4096] row-tile (Pool ucode only implements TT/TS, so
scan / scalar_tensor_tensor / tensor_tensor_reduce all live on DVE):
  DVE   : d[:, :DL] (TT sub), scan, thr, t1m (STT), loss-accum (STT+accum)
  GPSIMD: d[:, DL:] (TT sub), lp = labels_f32 * pre (TT)
  ACT   : rr = lp + 0.065 (activation copy+bias)
  loss_i = sum_j t1m * rr, chained across tiles via TTR initial.
  labels are cast int32->f32 during DMA (SWDGE); HBM traffic unchanged.
"""

import os
from contextlib import ExitStack

import numpy as np

B, L = 4096, 4096
N_CORES = 8
ROWS_PER_CORE = B // N_CORES          # 512
P = 128                               # partitions per tile
TILES = ROWS_PER_CORE // P            # 4
C_CONST = 0.65 * 0.1                  # 0.065
BIG = 1e30

_CACHE = {}


def _build_nc(repeat: int = 1, dl: int = 1408):
    import concourse.mybir as mybir
    import concourse.tile as tile
    from concourse import bacc

    f32 = mybir.dt.float32
    bf16 = mybir.dt.bfloat16
    i32 = mybir.dt.int32
    Op = mybir.AluOpType

    # Bacc (not raw Bass): its compile() runs generate_event_semaphores,
    # which splits multi-sem waits into standalone EventSemaphore
    # instructions (HW allows at most 1 wait per compute instruction).
    nc = bacc.Bacc("TRN2", target_bir_lowering=False, debug=False)

    out_d = nc.dram_tensor("out", [ROWS_PER_CORE, L * 2], f32, kind="ExternalInput")
    lab_d = nc.dram_tensor("lab", [ROWS_PER_CORE, L], i32, kind="ExternalInput")
    pre_d = nc.dram_tensor("pre", [P, L], f32, kind="ExternalInput")
    res_d = nc.dram_tensor("res", [P, 1], f32, kind="ExternalOutput")

    out_t = out_d[:].rearrange("(n p) m -> n p m", p=P)   # [4, 128, 8192]
    lab_t = lab_d[:].rearrange("(n p) m -> n p m", p=P)   # [4, 128, 4096]

    with tile.TileContext(nc) as tc, ExitStack() as ctx:
        io_pool = ctx.enter_context(tc.tile_pool(name="io", bufs=2))
        pre_pool = ctx.enter_context(tc.tile_pool(name="pre", bufs=1))
        d_pool = ctx.enter_context(tc.tile_pool(name="d", bufs=1))
        m_pool = ctx.enter_context(tc.tile_pool(name="m", bufs=1))
        t1m_pool = ctx.enter_context(tc.tile_pool(name="t1m", bufs=2))
        lp_pool = ctx.enter_context(tc.tile_pool(name="lp", bufs=1))
        rr_pool = ctx.enter_context(tc.tile_pool(name="rr", bufs=1))
        acc_pool = ctx.enter_context(tc.tile_pool(name="acc", bufs=1))

        pre_tl = pre_pool.tile([P, L], f32)
        nc.sync.dma_start(pre_tl[:], pre_d[:])

        acc_B = acc_pool.tile([P, TILES], f32)

        for _r in range(repeat):
            for k in range(TILES):
                ot = io_pool.tile([P, L * 2], f32, tag="ot")
                nc.sync.dma_start(ot[:], out_t[k])
                # labels: int32 -> f32 cast during DMA (SWDGE path); Pool
                # rejects int32 operands and HBM read traffic is unchanged.
                lt = io_pool.tile([P, L], f32, tag="lt")
                nc.gpsimd.dma_start(lt[:], lab_t[k])

                x3 = ot[:].rearrange("p (l c) -> p l c", c=2)
                t0 = x3[:, :, 0]
                t1 = x3[:, :, 1]

                # pass 1 (split DVE/Pool): d = t0 - t1
                d = d_pool.tile([P, L], f32)
                nc.vector.tensor_tensor(
                    d[:, 0:dl], t0[:, 0:dl], t1[:, 0:dl], Op.subtract)
                nc.gpsimd.tensor_tensor(
                    d[:, dl:L], t0[:, dl:L], t1[:, dl:L], Op.subtract)

                # pass 2 (DVE): M[j] = max(d[j:], -1), M[L] = -1 (bf16)
                M = m_pool.tile([P, L + 1], bf16)
                nc.vector.memset(M[:, L:L + 1], -1.0)
                nc.vector.tensor_tensor_scan(
                    M[:, 0:L][:, ::-1], d[:, ::-1], d[:, ::-1], -1.0,
                    Op.max, Op.max,
                )

                # tiny (DVE): thr = 0 if M[0] >= 0 else -BIG
                thr = acc_pool.tile([P, 1], f32, tag="thr")
                nc.vector.tensor_scalar(
                    thr[:], M[:, 0:1], 0.0, BIG, Op.is_ge, Op.mult
                )
                nc.vector.tensor_scalar_add(thr[:], thr[:], -BIG)

                # pass 3 (DVE): t1m = (M[j+1] >= thr) * t1
                t1m = t1m_pool.tile([P, L], f32)
                nc.vector.scalar_tensor_tensor(
                    t1m[:], M[:, 1:L + 1], thr[:], t1,
                    Op.is_ge, Op.mult,
                )

                # pass 4 (GPSIMD): lp = lab_f32 * pre
                lp = lp_pool.tile([P, L], f32)
                nc.gpsimd.tensor_tensor(lp[:], lt[:], pre_tl[:], Op.mult)

                # pass 5 (ACT): rr = lp + 0.065
                rr = rr_pool.tile([P, L], f32)
                nc.scalar.activation(
                    rr[:], lp[:],
                    mybir.ActivationFunctionType.Copy,
                    bias=C_CONST, scale=1.0,
                )

                # pass 6 (DVE): loss_k = sum(t1m * rr)
                # (tensor_tensor_reduce crashes TRN2 HW; STT+accum_out works)
                # Main output written in-place into t1m (1:1 streaming) so d
                # is not written here — otherwise Pool's next d-half would
                # serialize against this op (WAR), ping-ponging the engines.
                nc.vector.scalar_tensor_tensor(
                    t1m[:], t1m[:], 1.0, rr[:], Op.mult, Op.mult,
                    accum_out=acc_B[:, k:k + 1],
                )

            # tail: loss_i = sum_k loss_k
            loss_t = acc_pool.tile([P, 1], f32, tag="loss")
            nc.vector.reduce_sum(loss_t[:], acc_B[:], axis=mybir.AxisListType.X)

        nc.sync.dma_start(res_d[:], loss_t[:])

    nc.compile()
    return nc


def _pre_tile() -> np.ndarray:
    j = np.arange(L, dtype=np.float64)
    pre2 = (-3.6 / np.log2(j + 2.0) - C_CONST).astype(np.float32)
    return np.ascontiguousarray(np.tile(pre2[None, :], (P, 1)))


def _get_nc(repeat: int = 1):
    key = repeat
    if key not in _CACHE:
        _CACHE[key] = _build_nc(repeat=repeat)
    return _CACHE[key]


def make_in_maps(output: np.ndarray, labels: np.ndarray):
    pre = _pre_tile()
    in_maps = []
    for c in range(N_CORES):
        sl = slice(c * ROWS_PER_CORE, (c + 1) * ROWS_PER_CORE)
        in_maps.append({
            "out": np.ascontiguousarray(output[sl]).reshape(ROWS_PER_CORE, L * 2),
            "lab": np.ascontiguousarray(labels[sl]),
            "pre": pre,
        })
    return in_maps


def kernel(output: np.ndarray, labels: np.ndarray) -> np.ndarray:
    from concourse.bass_utils import run_bass_kernel_spmd

    nc = _get_nc(repeat=1)
    in_maps = make_in_maps(output, labels)
    r = run_bass_kernel_spmd(nc, in_maps, core_ids=list(range(N_CORES)))
    total = 0.0
    for res in r.results:
        total += float(res["res"].astype(np.float64).sum())
    return np.float32(total / B)


if __name__ == "__main__":
    # quick standalone run (full inputs, random)
    rng = np.random.default_rng(0)
    out = rng.standard_normal((B, L, 2)).astype(np.float32)
    lab = rng.integers(0, 2, size=(B, L)).astype(np.int32)
    print("loss:", kernel(out, lab))

